# revision 1
# baseline (speedup 1.0000x reference)
"""Decoder block (pre-norm attention + FFN) on 8 TRN2 NeuronCores, v3.

Sharding: batch-parallel (4 batch elements x 2 cores). Within a pair, queries
are split in causally-balanced 256-col blocks: core j=0 owns global q-blocks
[7,5,2,0], j=1 owns [6,4,3,1]; per-position key-chunk counts are padded to
fixed sizes [16,12,8,4] so one SPMD program serves both cores. Causality and
padding are enforced by ADDITIVE bf16 masks (0 / -1e4) accumulated into the
score PSUM via identity matmuls, on only the last two kc-pairs per position.

LayerNorm folds into the projections: mu/std per token come from ones-column
PE matmuls; Q = rstd*(x@W) - rstd*mu*csum(W) + b is a K=2 f32r fixup matmul
([csumW*s; b*s] x [-mu; std]) appended to the projection PSUM group, then an
eviction multiply by a broadcast rstd row. LN affine folds into weights
host-side.

Attention-side matmuls run fp8e4 DoubleRow (two k-tiles per pass, 0.5
cyc/col in the perf model): QKV and AV naive-fp8; scores via dk-split [32,2]
layouts (KT_s/QT_s built by partition-shift DMAs); Wo as an error-compensated
hi+lo fp8 pair. The FFN runs in bf16 (fp8 activations there cost too much
accuracy). Stats/fixups stay f32r. PSUM note: start=True zeroes a whole 2KB
bank, so co-resident accumulation groups (S quadrants, even/odd AV) let only
the first group in a bank use start=True.

The attention inner loop is software-pipelined (S/mask/exp of pair i overlap
AV of pair i-1) so the PE queue never head-of-line blocks on the ACT exp.
"""

import numpy as np
import ml_dtypes

B, T, D = 4, 2048, 768
H, DK, DFF = 12, 64, 3072
DC = D // 128          # 6 chunks of d_model
FC = DFF // 128        # 24 chunks of d_ff
EPS = 1e-5
NCORES = 8
QCOLS = 1024
QBLK = 256
CBLK = 512             # streaming chunk for stats/projections
NKC = T // 128         # 16 key chunks
POS_S = (4, 8, 12, 16)         # padded kc chunks per position
POS_P = (2, 4, 6, 8)           # kc pairs per position
BLOCKS = {0: (0, 2, 5, 7), 1: (1, 3, 4, 6)}
VAW = 208              # VA per-m block: even head cols 0:65, odd 80:208

_cache = {}

F8 = ml_dtypes.float8_e4m3


def _q8(w):
    s = 224.0 / max(float(np.abs(w).max()), 1e-30)
    return (w * s).astype(F8), s


def _q8_pair(w):
    s = 224.0 / max(float(np.abs(w).max()), 1e-30)
    hi = (w * s).astype(F8)
    lo = (w * s - hi.astype(np.float32)).astype(F8)
    return hi, lo, s


def _build(scales):
    import concourse.bacc as bacc
    import concourse.tile as tile
    import concourse.mybir as mybir
    from contextlib import ExitStack

    dt = mybir.dt
    F = mybir.ActivationFunctionType
    OP = mybir.AluOpType
    DR = mybir.MatmulPerfMode.DoubleRow

    s_q, s_k, s_v, s_o, s_1, s_2 = scales

    nc = bacc.Bacc("TRN2", target_bir_lowering=False, debug=False)

    # ---- DRAM I/O ----
    xtf = nc.dram_tensor("xtf", [128, DC, T], dt.bfloat16, kind="ExternalInput")
    xtq = nc.dram_tensor("xtq", [128, DC, QCOLS], dt.bfloat16, kind="ExternalInput")
    xsf = nc.dram_tensor("xsf", [128, DC, T], dt.bfloat16, kind="ExternalInput")
    xsq_d = nc.dram_tensor("xsq", [128, DC, QCOLS], dt.bfloat16, kind="ExternalInput")
    xtf8 = nc.dram_tensor("xtf8", [128, DC, T], dt.float8e4, kind="ExternalInput")
    xtq8 = nc.dram_tensor("xtq8", [128, DC, QCOLS], dt.float8e4, kind="ExternalInput")
    wq8_d = nc.dram_tensor("wq8", [128, DC, D], dt.float8e4, kind="ExternalInput")
    wk8_d = nc.dram_tensor("wk8", [128, DC, D], dt.float8e4, kind="ExternalInput")
    wv8_d = nc.dram_tensor("wv8", [128, DC, D], dt.float8e4, kind="ExternalInput")
    wo8h_d = nc.dram_tensor("wo8h", [128, DC, D], dt.float8e4, kind="ExternalInput")
    wo8l_d = nc.dram_tensor("wo8l", [128, DC, D], dt.float8e4, kind="ExternalInput")
    w1b_d = nc.dram_tensor("w1b", [128, DC, DFF], dt.bfloat16, kind="ExternalInput")
    w2b_d = nc.dram_tensor("w2b", [128, FC, D], dt.bfloat16, kind="ExternalInput")
    cqb_d = nc.dram_tensor("cqb", [2, D], dt.float32r, kind="ExternalInput")
    ckb_d = nc.dram_tensor("ckb", [2, D], dt.float32r, kind="ExternalInput")
    cvb_d = nc.dram_tensor("cvb", [2, D], dt.float32r, kind="ExternalInput")
    bos_d = nc.dram_tensor("bos", [1, D], dt.float32r, kind="ExternalInput")
    b2s_d = nc.dram_tensor("b2s", [1, D], dt.float32r, kind="ExternalInput")
    b1_d = nc.dram_tensor("b1", [128, FC], dt.float32, kind="ExternalInput")
    masks_d = nc.dram_tensor("masks", [128, 4, 2, 2, QBLK], dt.bfloat16,
                             kind="ExternalInput")
    ident_d = nc.dram_tensor("ident", [128, 128], dt.bfloat16, kind="ExternalInput")
    sel2_d = nc.dram_tensor("sel2", [2, 2], dt.float32, kind="ExternalInput")
    y_d = nc.dram_tensor("y", [128, DC, QCOLS], dt.float32r, kind="ExternalOutput")

    with tile.TileContext(nc) as tc, ExitStack() as ctx, \
            nc.allow_low_precision(reason="fp8/bf16 evictions are intentional"):
        const = ctx.enter_context(tc.tile_pool(name="const", bufs=1))
        onesF = const.tile([128, 512], dt.float32)
        nc.vector.memset(onesF[:], 1.0)
        ones2 = const.tile([128, 2], dt.float32r)      # stats lhsT [K=128, M=2]
        nc.vector.tensor_copy(ones2[:], onesF[:, 0:2])
        ones2b = const.tile([128, 2], dt.bfloat16)     # stats lhsT for bf16 rhs
        nc.vector.tensor_copy(ones2b[:], onesF[:, 0:2])
        ones_row = const.tile([1, 128], dt.float32r)   # K=1 bcast lhsT [1, 128]
        nc.vector.tensor_copy(ones_row[:], onesF[0:1, 0:128])
        ones512 = const.tile([1, 512], dt.float32r)
        nc.vector.tensor_copy(ones512[:], onesF[0:1, :])
        eps_t = const.tile([2, 1], dt.float32)
        nc.vector.memset(eps_t[:], EPS)
        cqb = const.tile([2, D], dt.float32r)
        ckb = const.tile([2, D], dt.float32r)
        cvb = const.tile([2, D], dt.float32r)
        bos = const.tile([1, D], dt.float32r)
        b2s = const.tile([1, D], dt.float32r)
        b1_t = const.tile([128, FC], dt.float32)
        ident = const.tile([128, 128], dt.bfloat16)
        sel2 = const.tile([2, 2], dt.float32)          # [[-1,0],[0,1]] columns
        ones_c64 = const.tile([65, 64], dt.float32r)   # rows 0 & 64 used as [1,64]
        nc.vector.tensor_copy(ones_c64[:], onesF[0:65, 0:64])
        nc.sync.dma_start(cqb[:], cqb_d[:])
        nc.sync.dma_start(ckb[:], ckb_d[:])
        nc.sync.dma_start(cvb[:], cvb_d[:])
        nc.sync.dma_start(bos[:], bos_d[:])
        nc.sync.dma_start(b2s[:], b2s_d[:])
        nc.sync.dma_start(b1_t[:], b1_d[:])
        nc.sync.dma_start(ident[:], ident_d[:])
        nc.sync.dma_start(sel2[:], sel2_d[:])

        # persistent attention-side tensors (live phases 1-3)
        pBIG = tc.alloc_tile_pool(name="big", bufs=1)
        # dk-split, partition-packed: m<3 at partitions 0-63, m>=3 at 64-127;
        # within a half: head h at base 32h (h=0/1 -> 0/32, +64 for m>=3)
        KTs = pBIG.tile([128, 3, 2, T], dt.float8e4)
        QTs = pBIG.tile([128, 3, 2, QCOLS], dt.float8e4)
        VA = pBIG.tile([128, NKC, DC, VAW], dt.float8e4)
        mk = pBIG.tile([128, 4, 2, 2, QBLK], dt.bfloat16)
        AT8 = pBIG.tile([128, DC, QCOLS], dt.float8e4)
        rcol = pBIG.tile([128, NKC], dt.float32)       # rstd col per 128-chunk
        nc.vector.tensor_copy(VA[:, :, :, 64:65],
                              onesF[:, 0:1].to_broadcast([128, NKC, DC, 1]))
        for _c in (65, 66, 67):
            nc.vector.tensor_copy(VA[:, :, :, _c:_c + 1],
                                  onesF[:, 0:1].to_broadcast([128, NKC, DC, 1]))
        nc.vector.tensor_copy(VA[:, :, :, 80:81],
                              onesF[:, 0:1].to_broadcast([128, NKC, DC, 1]))

        # phase-1 transients (released after phase 1b)
        pKTQ = tc.alloc_tile_pool(name="ktq", bufs=1)
        KT = pKTQ.tile([128, DC, T], dt.float8e4)
        QT = pKTQ.tile([128, DC, QCOLS], dt.float8e4)
        mst = pKTQ.tile([2, T], dt.float32r)           # [-mu; std] rows
        mstq = pKTQ.tile([2, QCOLS], dt.float32r)
        rrow = pKTQ.tile([1, T], dt.float32r)          # rstd row
        rrowq = pKTQ.tile([1, QCOLS], dt.float32r)
        rrow_v = pKTQ.tile([1, T], dt.float32r)        # rstd / s_v row
        wq8 = pKTQ.tile([128, DC, D], dt.float8e4)

        def stats_rows(xt_cb, sq_cb, ncols, psum, pool, mst_t, rrow_t, off,
                       ones_x=None):
            """LN stats over partitions -> [-mu; std] rows + rstd row."""
            ox = ones_x if ones_x is not None else ones2b
            s_ps = psum.tile([2, ncols], dt.float32, tag="lnS")
            q_ps = psum.tile([2, ncols], dt.float32, tag="lnQ")
            for c in range(DC):
                nc.tensor.matmul(s_ps[:], ox[:], xt_cb[:, c, :],
                                 start=(c == 0), stop=(c == DC - 1))
                nc.tensor.matmul(q_ps[:], ones2b[:], sq_cb[:, c, :],
                                 start=(c == 0), stop=(c == DC - 1))
            mu = pool.tile([2, ncols], dt.float32, tag="lnmu")
            nc.vector.tensor_scalar_mul(mu[:], s_ps[:], 1.0 / D)
            var = pool.tile([2, ncols], dt.float32, tag="lnvar")
            nc.vector.scalar_tensor_tensor(
                out=var[:], in0=mu[:], scalar=-1.0, in1=mu[:],
                op0=OP.mult, op1=OP.mult)
            nc.vector.scalar_tensor_tensor(
                out=var[:], in0=q_ps[:], scalar=1.0 / D, in1=var[:],
                op0=OP.mult, op1=OP.add)
            std = pool.tile([2, ncols], dt.float32, tag="lnstd")
            nc.scalar.activation(out=std[:], in_=var[:], func=F.Sqrt, bias=eps_t[:])
            cs = slice(off, off + ncols)
            t = pool.tile([2, ncols], dt.float32, tag="lnt")
            nc.vector.tensor_scalar_mul(t[:], mu[:], sel2[:, 0:1])
            nc.vector.scalar_tensor_tensor(
                out=mst_t[:, cs], in0=std[:], scalar=sel2[:, 1:2], in1=t[:],
                op0=OP.mult, op1=OP.add)
            nc.vector.reciprocal(rrow_t[0:1, cs], std[0:1, :])

        # ---- Phase 1: stats + K/V over full T (4 chunks of 512) ----
        with (
            tc.tile_pool(name="p1", bufs=2) as p1,
            tc.tile_pool(name="p1r", bufs=2) as p1r,
            tc.tile_pool(name="p1w", bufs=1) as p1w,
            tc.tile_pool(name="p1ps", bufs=2, space="PSUM") as ps1,
            tc.tile_pool(name="p1ls", bufs=1, space="PSUM") as psl1,
        ):
            wk8 = p1w.tile([128, DC, D], dt.float8e4)
            wv8 = p1w.tile([128, DC, D], dt.float8e4)
            nc.sync.dma_start(wk8[:], wk8_d[:])
            nc.sync.dma_start(wv8[:], wv8_d[:])
            for cb in range(T // CBLK):
                cs = slice(cb * CBLK, (cb + 1) * CBLK)
                xt = p1.tile([128, DC, CBLK], dt.bfloat16, tag="xt")
                nc.sync.dma_start(xt[:], xtf[:, :, cs])
                x8 = p1.tile([128, DC, CBLK], dt.float8e4, tag="x8")
                nc.sync.dma_start(x8[:], xtf8[:, :, cs])
                sq = p1.tile([128, DC, CBLK], dt.bfloat16, tag="sq")
                nc.sync.dma_start(sq[:], xsf[:, :, cs])
                if cb == 1:
                    nc.sync.dma_start(wq8[:], wq8_d[:])
                    nc.sync.dma_start(mk[:], masks_d[:])
                stats_rows(xt, sq, CBLK, psl1, p1r, mst, rrow, cb * CBLK)
                r_ps = psl1.tile([128, CBLK], dt.float32, tag="rps")
                nc.tensor.matmul(r_ps[:], ones_row[:], rrow[0:1, cs],
                                 start=True, stop=True)
                r_sb = p1r.tile([128, CBLK], dt.float32, tag="rsb")
                nc.scalar.copy(r_sb[:], r_ps[:])
                nc.vector.tensor_scalar_mul(rrow_v[0:1, cs], rrow[0:1, cs],
                                            1.0 / s_v)
                for rc in range(4):
                    idx = cb * 4 + rc
                    nc.gpsimd.dma_start(
                        rcol[:, idx:idx + 1],
                        rrow_v[0:1, idx * 128:(idx + 1) * 128]
                        .bitcast(dt.float32))
                # K projection (DR first, fixup last)
                for m in range(DC):
                    kps = ps1.tile([128, CBLK], dt.float32, tag="kps")
                    for j in range(3):
                        nc.tensor.matmul(
                            kps[:], wk8[:, 2 * j:2 * j + 2, m * 128:(m + 1) * 128],
                            x8[:, 2 * j:2 * j + 2, :],
                            start=(j == 0), stop=False, perf_mode=DR)
                    nc.tensor.matmul(kps[:], ckb[:, m * 128:(m + 1) * 128],
                                     mst[:, cs], start=False, stop=True)
                    nc.vector.scalar_tensor_tensor(
                        out=KT[:, m, cs], in0=kps[:], scalar=1.0 / s_k,
                        in1=r_sb[:], op0=OP.mult, op1=OP.mult)
                # V projection (row-major, 128-token grain)
                for rc in range(CBLK // 128):
                    rs = slice(rc * 128, (rc + 1) * 128)
                    kc_idx = cb * 4 + rc
                    for nh in range(2):
                        ns = slice(nh * 384, (nh + 1) * 384)
                        vps = ps1.tile([128, 384], dt.float32, tag="vps")
                        for j in range(3):
                            nc.tensor.matmul(
                                vps[:], x8[:, 2 * j:2 * j + 2, rs],
                                wv8[:, 2 * j:2 * j + 2, ns],
                                start=(j == 0), stop=False, perf_mode=DR)
                        nc.tensor.matmul(
                            vps[:], mst[:, cb * CBLK + rc * 128:
                                        cb * CBLK + rc * 128 + 128],
                            cvb[:, ns], start=False, stop=True)
                        src4 = vps[:].rearrange(
                            "p (hp par d) -> p hp par d", par=2, d=64)
                        for par in range(2):
                            nc.scalar.activation(
                                out=VA[:, kc_idx, 3 * nh:3 * nh + 3,
                                       par * 144:par * 144 + 64],
                                in_=src4[:, :, par, :], func=F.Copy,
                                scale=rcol[:, kc_idx:kc_idx + 1])
                # dk-split remap of this K chunk (partition shifts)
                for h in range(2):
                    for s in range(2):
                        for mh in range(2):
                            nc.sync.dma_start(
                                KTs[64 * mh + 32 * h:64 * mh + 32 * h + 32,
                                    :, s, cs],
                                KT[64 * h + 32 * s:64 * h + 32 * s + 32,
                                   3 * mh:3 * mh + 3, cs])

        # ---- Phase 1b: stats + Q over own q columns (2 chunks of 512) ----
        with (
            tc.tile_pool(name="p2", bufs=2) as p2,
            tc.tile_pool(name="p2r", bufs=2) as p2r,
            tc.tile_pool(name="p2ps", bufs=2, space="PSUM") as ps2,
            tc.tile_pool(name="p2ls", bufs=1, space="PSUM") as psl2,
        ):
            for cb in range(QCOLS // CBLK):
                cs = slice(cb * CBLK, (cb + 1) * CBLK)
                xt = p2.tile([128, DC, CBLK], dt.bfloat16, tag="xt")
                nc.sync.dma_start(xt[:], xtq[:, :, cs])
                x8 = p2.tile([128, DC, CBLK], dt.float8e4, tag="x8")
                nc.sync.dma_start(x8[:], xtq8[:, :, cs])
                sq = p2.tile([128, DC, CBLK], dt.bfloat16, tag="sq")
                nc.sync.dma_start(sq[:], xsq_d[:, :, cs])
                stats_rows(xt, sq, CBLK, psl2, p2r, mstq, rrowq, cb * CBLK)
                r_ps = psl2.tile([128, CBLK], dt.float32, tag="rps")
                nc.tensor.matmul(r_ps[:], ones_row[:], rrowq[0:1, cs],
                                 start=True, stop=True)
                r_sb = p2r.tile([128, CBLK], dt.float32, tag="rsb")
                nc.scalar.copy(r_sb[:], r_ps[:])
                for m in range(DC):
                    qps = ps2.tile([128, CBLK], dt.float32, tag="qps")
                    for j in range(3):
                        nc.tensor.matmul(
                            qps[:], wq8[:, 2 * j:2 * j + 2, m * 128:(m + 1) * 128],
                            x8[:, 2 * j:2 * j + 2, :],
                            start=(j == 0), stop=False, perf_mode=DR)
                    nc.tensor.matmul(qps[:], cqb[:, m * 128:(m + 1) * 128],
                                     mstq[:, cs], start=False, stop=True)
                    nc.vector.scalar_tensor_tensor(
                        out=QT[:, m, cs], in0=qps[:], scalar=1.0 / s_q,
                        in1=r_sb[:], op0=OP.mult, op1=OP.mult)
                for h in range(2):
                    for s in range(2):
                        for mh in range(2):
                            nc.sync.dma_start(
                                QTs[64 * mh + 32 * h:64 * mh + 32 * h + 32,
                                    :, s, cs],
                                QT[64 * h + 32 * s:64 * h + 32 * s + 32,
                                   3 * mh:3 * mh + 3, cs])
        pKTQ.release()

        # FFN/Wo weights + x2 on the right side; DMAs overlap attention
        pR = tc.alloc_tile_pool(name="pr", bufs=1, side="right")
        x2 = pR.tile([128, DC, QCOLS], dt.float32r)
        wo8h = pR.tile([128, DC, D], dt.float8e4)
        wo8l = pR.tile([128, DC, D], dt.float8e4)
        w2b = pR.tile([128, FC, D], dt.bfloat16)
        xqs = []
        for cb in range(2):
            xq = pR.tile([128, DC, 512], dt.bfloat16)
            nc.sync.dma_start(xq[:], xtq[:, :, cb * 512:(cb + 1) * 512])
            xqs.append(xq)
        nc.sync.dma_start(wo8h[:], wo8h_d[:])
        nc.sync.dma_start(wo8l[:], wo8l_d[:])
        nc.sync.dma_start(w2b[:], w2b_d[:])

        # ---- Phase 2: attention (software-pipelined) ----
        with (
            tc.tile_pool(name="cl", bufs=4) as pc,
            tc.tile_pool(name="cn", bufs=2) as pn,
            tc.tile_pool(name="cps", bufs=2, space="PSUM") as psc,
            tc.tile_pool(name="cpo", bufs=2, space="PSUM") as pso,
        ):
            for pos in range(4):
                qs = slice(pos * QBLK, (pos + 1) * QBLK)
                npair = POS_P[pos]
                for m in range(DC):
                    oall = pso.tile([128, 3, QBLK], dt.float32, tag="oall")
                    o_e = oall[:, 0, :]
                    o_o = oall[:, 1, :]
                    pmms = []
                    for pr in range(npair):
                        masked = pr >= npair - 2
                        sps = psc.tile([128, 2, 2, QBLK], dt.float32, tag="sps")
                        for h in range(2):
                            b0 = 64 * (m // 3) + 32 * h
                            for kk in range(2):
                                kc = 2 * pr + kk
                                nc.tensor.matmul(
                                    sps[:, h, kk, :],
                                    KTs[b0:b0 + 32, m % 3, :,
                                        kc * 128:(kc + 1) * 128],
                                    QTs[b0:b0 + 32, m % 3, :, qs],
                                    start=(kk == 0), stop=not masked,
                                    perf_mode=DR, skip_group_check=True,
                                    tile_position=(b0, 0))
                            if masked:
                                slot = pr - (npair - 2)
                                nc.tensor.matmul(
                                    sps[:, h, :, :], ident[:],
                                    mk[:, pos, slot, :, :],
                                    start=False, stop=True,
                                    skip_group_check=True)
                        pmm = pc.tile([128, 2, 2, QBLK], dt.float8e4, tag="pmm")
                        nc.scalar.activation(out=pmm[:], in_=sps[:], func=F.Exp)
                        pmms.append(pmm)
                        if pr > 0:
                            q = pmms[pr - 1]
                            nc.tensor.matmul(
                                o_e[0:68, :], VA[:, 2 * pr - 2:2 * pr, m, 0:68],
                                q[:, 0, :, :],
                                start=(pr == 1), stop=False, perf_mode=DR)
                            nc.tensor.matmul(
                                o_o[:, :], VA[:, 2 * pr - 2:2 * pr, m, 80:208],
                                q[:, 1, :, :],
                                start=False, stop=False, perf_mode=DR,
                                skip_group_check=True)
                    q = pmms[npair - 1]
                    nc.tensor.matmul(
                        o_e[0:68, :], VA[:, 2 * npair - 2:2 * npair, m, 0:68],
                        q[:, 0, :, :],
                        start=(npair == 1), stop=True, perf_mode=DR)
                    nc.tensor.matmul(
                        o_o[:, :], VA[:, 2 * npair - 2:2 * npair, m, 80:208],
                        q[:, 1, :, :],
                        start=False, stop=True, perf_mode=DR,
                        skip_group_check=True)
                    # reciprocal of both denominators (odd @p0, even @p64),
                    # broadcast via column-tiled K=1 matmuls into one psum
                    dn = pn.tile([128, QBLK], dt.float32r, tag="dn")
                    nc.vector.reciprocal(dn[0:1, :], o_o[0:1, :])
                    nc.vector.reciprocal(dn[64:65, :], o_e[64:65, :])
                    # even head: PE-broadcast 1/denom (p64 row) to psum p0:64
                    rr_ps = oall[0:64, 2, :]
                    nc.tensor.matmul(rr_ps, ones_c64[64:65, :],
                                     dn[64:65, :], start=True, stop=True)
                    rr_sb = pn.tile([128, QBLK], dt.float32, tag="rrsb")
                    nc.vector.tensor_copy(rr_sb[0:64, :], rr_ps)
                    nc.vector.tensor_mul(AT8[0:64, m, qs], o_e[0:64, :],
                                         rr_sb[0:64, :])
                    # odd head: Pool broadcast 1/denom (p0 row) to all parts
                    rr_o = pn.tile([128, QBLK], dt.float32r, tag="rro")
                    nc.gpsimd.partition_broadcast(rr_o[:], dn[0:1, :])
                    nc.vector.tensor_mul(AT8[64:128, m, qs], o_o[64:128, :],
                                         rr_o[64:128, :])

        # ---- Phase 3: Wo + residual ----
        with (
            tc.tile_pool(name="dl", bufs=2) as pd,
            tc.tile_pool(name="dps", bufs=2, space="PSUM") as psd,
        ):
            for cb in range(2):
                cs = slice(cb * 512, (cb + 1) * 512)
                xq = xqs[cb]
                for m in range(DC):
                    ms = slice(m * 128, (m + 1) * 128)
                    ops = psd.tile([128, 512], dt.float32, tag="ops")
                    for j in range(3):
                        pj = slice(2 * j, 2 * j + 2)
                        nc.tensor.matmul(ops[:], wo8h[:, pj, ms], AT8[:, pj, cs],
                                         start=(j == 0), stop=False, perf_mode=DR)
                        nc.tensor.matmul(ops[:], wo8l[:, pj, ms], AT8[:, pj, cs],
                                         start=False, stop=False, perf_mode=DR)
                    nc.tensor.matmul(ops[:], bos[:, ms], ones512[:],
                                     start=False, stop=True)
                    nc.vector.scalar_tensor_tensor(
                        out=x2[:, m, cs], in0=ops[:], scalar=1.0 / s_o,
                        in1=xq[:, m, :], op0=OP.mult, op1=OP.add)
        pBIG.release()

        # ---- Phase 4: LN2 + FFN ----
        with (
            tc.tile_pool(name="el", bufs=2) as pe,
            tc.tile_pool(name="er", bufs=2) as per,
            tc.tile_pool(name="eh", bufs=1) as phh,
            tc.tile_pool(name="fw", bufs=1) as pf,
            tc.tile_pool(name="els", bufs=1, space="PSUM") as psl4,
            tc.tile_pool(name="fps", bufs=2, space="PSUM") as psa,
            tc.tile_pool(name="fps2", bufs=2, space="PSUM") as psy,
        ):
            mst2 = phh.tile([2, QCOLS], dt.float32r)
            rrow2 = phh.tile([1, QCOLS], dt.float32r)
            h2 = phh.tile([128, DC, QCOLS], dt.bfloat16)
            h1 = phh.tile([128, FC, QCOLS], dt.bfloat16)

            for cb in range(QCOLS // CBLK):
                cs = slice(cb * CBLK, (cb + 1) * CBLK)
                sq = pe.tile([128, DC, CBLK], dt.bfloat16, tag="sq")
                nc.gpsimd.tensor_mul(sq[:], x2[:, :, cs], x2[:, :, cs])
                stats_rows(x2[:, :, cs], sq, CBLK, psl4, per, mst2, rrow2,
                           cb * CBLK, ones_x=ones2)
                n_ps = psl4.tile([128, CBLK], dt.float32, tag="nps")
                nc.tensor.matmul(n_ps[:], ones_row[:], mst2[0:1, cs],
                                 start=True, stop=True)
                r_ps = psl4.tile([128, CBLK], dt.float32, tag="rps")
                nc.tensor.matmul(r_ps[:], ones_row[:], rrow2[0:1, cs],
                                 start=True, stop=True)
                for c in range(DC):
                    t1 = pe.tile([128, CBLK], dt.float32, tag="t1")
                    nc.vector.tensor_add(t1[:], x2[:, c, cs], n_ps[:])
                    nc.vector.tensor_mul(h2[:, c, cs], t1[:], r_ps[:])
            # W1 + gelu -> h1 (fp8), 3 ff-groups of 8 chunks
            for g in range(3):
                w1b = pf.tile([128, DC, 1024], dt.bfloat16, tag="w1b")
                nc.sync.dma_start(w1b[:], w1b_d[:, :, g * 1024:(g + 1) * 1024])
                for mf in range(8):
                    fc = g * 8 + mf
                    ms = slice(mf * 128, (mf + 1) * 128)
                    for cb in range(2):
                        cs = slice(cb * 512, (cb + 1) * 512)
                        aps = psa.tile([128, 512], dt.float32, tag="aps")
                        for c in range(DC):
                            nc.tensor.matmul(aps[:], w1b[:, c, ms], h2[:, c, cs],
                                             start=(c == 0), stop=(c == DC - 1))
                        nc.scalar.activation(
                            out=h1[:, fc, cs], in_=aps[:], func=F.Gelu,
                            bias=b1_t[:, fc:fc + 1])
            # W2 + residual
            for cb in range(2):
                cs = slice(cb * 512, (cb + 1) * 512)
                for m in range(DC):
                    ms = slice(m * 128, (m + 1) * 128)
                    yps = psy.tile([128, 512], dt.float32, tag="yps")
                    for fc in range(FC):
                        nc.tensor.matmul(yps[:], w2b[:, fc, ms], h1[:, fc, cs],
                                         start=(fc == 0), stop=False)
                    nc.tensor.matmul(yps[:], b2s[:, ms], ones512[:],
                                     start=False, stop=True)
                    nc.vector.scalar_tensor_tensor(
                        out=x2[:, m, cs], in0=yps[:], scalar=1.0,
                        in1=x2[:, m, cs], op0=OP.mult, op1=OP.add)
                nc.sync.dma_start(y_d[:, :, cs], x2[:, :, cs])
        pR.release()

    nc.compile()
    return nc


def _to_lhsT(w):
    """[Din, Dout] -> [128, Din//128, Dout] partition-chunked lhsT layout."""
    din, dout = w.shape
    return np.ascontiguousarray(w.reshape(din // 128, 128, dout).transpose(1, 0, 2))


def _to_tposed(xb):
    """[T?, 768] -> [128, 6, T?] transposed chunked layout."""
    t = xb.shape[0]
    return np.ascontiguousarray(xb.T.reshape(DC, 128, t).transpose(1, 0, 2))


def kernel(**inputs):
    from concourse.bass_utils import run_bass_kernel_spmd

    x = np.asarray(inputs["x"], np.float32)
    ln1_g = np.asarray(inputs["ln1_g"], np.float32)
    ln1_b = np.asarray(inputs["ln1_b"], np.float32)
    ln2_g = np.asarray(inputs["ln2_g"], np.float32)
    ln2_b = np.asarray(inputs["ln2_b"], np.float32)
    Wq = np.asarray(inputs["Wq"], np.float32)
    bq = np.asarray(inputs["bq"], np.float32)
    Wk = np.asarray(inputs["Wk"], np.float32)
    bk = np.asarray(inputs["bk"], np.float32)
    Wv = np.asarray(inputs["Wv"], np.float32)
    bv = np.asarray(inputs["bv"], np.float32)
    Wo = np.asarray(inputs["Wo"], np.float32)
    bo = np.asarray(inputs["bo"], np.float32)
    W1 = np.asarray(inputs["W1"], np.float32)
    b1 = np.asarray(inputs["b1"], np.float32)
    W2 = np.asarray(inputs["W2"], np.float32)
    b2 = np.asarray(inputs["b2"], np.float32)

    sc = 1.0 / np.sqrt(np.float32(DK))
    Wq_ = (ln1_g[:, None] * Wq) * sc
    bq_ = (ln1_b @ Wq + bq) * sc
    Wk_ = ln1_g[:, None] * Wk
    bk_ = ln1_b @ Wk + bk
    Wv_ = ln1_g[:, None] * Wv
    bv_ = ln1_b @ Wv + bv
    W1_ = ln2_g[:, None] * W1
    b1_ = ln2_b @ W1 + b1

    wq8, s_q = _q8(Wq_)
    wk8, s_k = _q8(Wk_)
    wv8, s_v = _q8(Wv_)
    wo8h, wo8l, s_o = _q8_pair(Wo)
    s_1 = s_2 = 1.0
    scales = (s_q, s_k, s_v, s_o, s_1, s_2)

    key = tuple(np.float32(s) for s in scales)
    if key not in _cache:
        _cache.clear()
        _cache[key] = _build(scales)
    nc = _cache[key]

    def crow(w8_lhsT, s, bias):
        csum = w8_lhsT.astype(np.float32).sum(axis=(0, 1))
        return np.ascontiguousarray(
            np.stack([csum, s * bias]).astype(np.float32))

    wq8_l, wk8_l, wv8_l = _to_lhsT(wq8), _to_lhsT(wk8), _to_lhsT(wv8)
    shared = {
        "wq8": wq8_l, "wk8": wk8_l, "wv8": wv8_l,
        "wo8h": _to_lhsT(wo8h), "wo8l": _to_lhsT(wo8l),
        "w1b": _to_lhsT(W1_).astype(ml_dtypes.bfloat16),
        "w2b": _to_lhsT(W2).astype(ml_dtypes.bfloat16),
        "cqb": crow(wq8_l, s_q, bq_),
        "ckb": crow(wk8_l, s_k, bk_),
        "cvb": crow(wv8_l, s_v, bv_),
        "bos": (bo * s_o).reshape(1, D).astype(np.float32),
        "b2s": b2.reshape(1, D).astype(np.float32),
        "b1": np.ascontiguousarray(b1_.reshape(-1, 128).T).astype(np.float32),
        "ident": np.eye(128, dtype=np.float32).astype(ml_dtypes.bfloat16),
        "sel2": np.array([[-1.0, 0.0], [0.0, 1.0]], np.float32),
    }

    in_maps = []
    qcols_per_core = []
    for core in range(NCORES):
        b, j = core // 2, core % 2
        blocks = BLOCKS[j]
        qpos = np.concatenate([np.arange(bb * QBLK, (bb + 1) * QBLK)
                               for bb in blocks])
        qcols_per_core.append(qpos)
        masks = np.full((128, 4, 2, 2, QBLK), -10000.0, np.float32)
        part = np.arange(128)[:, None]
        for p in range(4):
            bb = blocks[p]
            S_p = POS_S[p]
            valid = 2 * (bb + 1)
            qg = np.arange(bb * QBLK, (bb + 1) * QBLK)[None, :]
            for slot in range(2):
                for kk in range(2):
                    chunk = S_p - 4 + 2 * slot + kk
                    if chunk < valid:
                        masks[:, p, slot, kk, :] = np.where(
                            (128 * chunk + part) <= qg, 0.0, -10000.0)
        m = dict(shared)
        xb = x[b]
        xtf_f = _to_tposed(xb)
        xtq_f = _to_tposed(xb[qpos])
        m["xtf"] = xtf_f.astype(ml_dtypes.bfloat16)
        m["xtq"] = xtq_f.astype(ml_dtypes.bfloat16)
        m["xsf"] = (xtf_f * xtf_f).astype(ml_dtypes.bfloat16)
        m["xsq"] = (xtq_f * xtq_f).astype(ml_dtypes.bfloat16)
        m["xtf8"] = xtf_f.astype(F8)
        m["xtq8"] = xtq_f.astype(F8)
        m["masks"] = masks.astype(ml_dtypes.bfloat16)
        in_maps.append(m)

    res = run_bass_kernel_spmd(nc, in_maps, core_ids=list(range(NCORES)))

    y = np.empty((B, T, D), np.float32)
    for core in range(NCORES):
        b = core // 2
        yt = res.results[core]["y"]                      # [128, DC, QCOLS]
        y[b, qcols_per_core[core]] = yt.transpose(1, 0, 2).reshape(D, QCOLS).T
    return y

